# revision 17
# baseline (speedup 1.0000x reference)
"""Trainium2 Bass kernel for nn_NetworkLayer_79173427134941 (gnn_message_passing).

Reference computation (per batch item b, N=1024 points, 3D coords):
    norms = ||x_b||                      [N, 1]
    dots  = sqrt(x_b @ x_b^T)            [N, N]
    scalars = [u_b (G=8) | norms | dots] [N, 1033]
    h = LeakyReLU(scalars @ W0 + b0); h = LeakyReLU(h @ W1 + b1)
    fk = h @ W2 + b2                     [N, 128]
    out_b = einsum('io,id->od', fk, x_b) / N    [128, 3]

Strategy:
  - Data-parallel over batch: 4 batch items per core x 8 cores.
  - Never materialize dots in HBM: gram on TensorE (fp32r), sqrt on ScalarE
    during PSUM->SBUF eviction, MLP fused on-chip in transposed [H, N] layout.
  - u-part + b0 folded into a host-precomputed K=2 rhs chunk [norms; ones].
  - Final contraction uses associativity:
       out_b^T = (x_b^T @ h1) @ W2 + b2 (x) colsum(x_b)
    so the device only returns y_b = x_b^T @ h1  [3, 128]; the last tiny
    [3,128]@[128,128] matmul + bias outer product runs on host.
"""

import numpy as np

B, N, G = 32, 1024, 8
H, K_OUT = 128, 128
N_CORES = 8
BPC = B // N_CORES  # batch items per core
NEG_SLOPE = 0.01

_cached = {}
TAIL_BF16 = True


def _build_nc(tail_bf16=TAIL_BF16):
    import concourse.bass as bass
    import concourse.tile as tile
    from concourse import bacc, mybir

    f32 = mybir.dt.float32
    f32r = mybir.dt.float32r
    bf16 = mybir.dt.bfloat16
    tdt = bf16 if tail_bf16 else f32r
    MUL = mybir.AluOpType.mult
    ADD = mybir.AluOpType.add

    nc = bacc.Bacc(
        "TRN2",
        target_bir_lowering=False,
        debug=False,
        enable_asserts=True,
        num_devices=N_CORES,
    )

    # DRAM I/O (per core)
    xT_d = nc.dram_tensor("xT", [BPC, 3, N], f32r, kind="ExternalInput").ap()
    xc_d = nc.dram_tensor("xc", [BPC, 128, 24], tdt, kind="ExternalInput").ap()
    rhs2_d = nc.dram_tensor("rhs2", [BPC, 2, N], f32r, kind="ExternalInput").ap()
    lw2_d = nc.dram_tensor("lw2", [BPC, 2, H], f32r, kind="ExternalInput").ap()
    w0d_d = nc.dram_tensor("w0d", [128, 1024], f32r, kind="ExternalInput").ap()
    w1_d = nc.dram_tensor("w1", [128, H], tdt, kind="ExternalInput").ap()
    b1t_d = nc.dram_tensor("b1t", [1, N], f32r, kind="ExternalInput").ap()
    ones_d = nc.dram_tensor("ones", [1, N], f32r, kind="ExternalInput").ap()
    y_d = nc.dram_tensor("y", [BPC, 3, H], f32, kind="ExternalOutput").ap()

    NCHUNK = N // 128  # 8 K-chunks of the dots matmul

    with tile.TileContext(nc) as tc:
        with (
            tc.tile_pool(name="const", bufs=1) as constp,
            tc.tile_pool(name="data", bufs=2) as datap,
            tc.tile_pool(name="dots", bufs=2) as dotsp,
            tc.tile_pool(name="act", bufs=2) as actp,
            tc.tile_pool(name="yout", bufs=2) as youtp,
            tc.tile_pool(name="gram", bufs=2, space="PSUM") as gramp,
            tc.tile_pool(name="h0p", bufs=1, space="PSUM") as h0pp,
            tc.tile_pool(name="h1p", bufs=1, space="PSUM") as h1pp,
        ):
            # const tiles (DMAs issued after batch-0 loads; see emit_consts)
            w0d_sb = constp.tile([128, 1024], f32r)
            w1_sb = constp.tile([128, H], tdt)
            b1t_sb = constp.tile([1, N], f32r)
            ones_sb = constp.tile([1, N], f32r)

            def emit_consts():
                nc.sync.dma_start(out=w0d_sb[:], in_=w0d_d[:])
                nc.sync.dma_start(out=w1_sb[:], in_=w1_d[:])
                nc.sync.dma_start(out=b1t_sb[:], in_=b1t_d[:])
                nc.sync.dma_start(out=ones_sb[:], in_=ones_d[:])

            state = {}

            def leaky_evict(out_ap, ps_ap, tmp_ap):
                # leaky(x) = 0.01*x + 0.99*relu(x); two ops so each reads PSUM once
                nc.vector.tensor_scalar(
                    tmp_ap, ps_ap, 0.0, 0.99, mybir.AluOpType.max, MUL
                )
                nc.vector.scalar_tensor_tensor(out_ap, ps_ap, 0.01, tmp_ap, MUL, ADD)

            def emit_gram(b):
                """DMA inputs for batch item b; gram matmuls + sqrt eviction."""
                xT_sb = datap.tile([3, N], f32r, tag="xT")
                nc.sync.dma_start(out=xT_sb[:], in_=xT_d[b])
                xc_sb = datap.tile([128, 24], tdt, tag="xc")
                nc.sync.dma_start(out=xc_sb[:], in_=xc_d[b])
                rhs2_sb = datap.tile([2, N], f32r, tag="rhs2")
                nc.sync.dma_start(out=rhs2_sb[:], in_=rhs2_d[b])
                lw2_sb = datap.tile([2, H], f32r, tag="lw2")
                nc.sync.dma_start(out=lw2_sb[:], in_=lw2_d[b])
                if b == 0:
                    emit_consts()

                dots_sb = dotsp.tile([128, NCHUNK * N], f32r, tag="dots")
                for m in range(NCHUNK):
                    g_ps = gramp.tile([128, N], f32, tag="g")
                    lhsT = xT_sb[:, 128 * m : 128 * (m + 1)]
                    for half in range(2):
                        nc.tensor.matmul(
                            g_ps[:, 512 * half : 512 * (half + 1)],
                            lhsT,
                            xT_sb[:, 512 * half : 512 * (half + 1)],
                            start=True,
                            stop=True,
                        )
                    # sqrt eviction PSUM -> SBUF strip m
                    nc.scalar.sqrt(dots_sb[:, N * m : N * (m + 1)], g_ps[:])
                state[b] = (xc_sb, rhs2_sb, lw2_sb, dots_sb)

            def emit_mlp(b):
                """MLP + output contraction for batch item b."""
                xc_sb, rhs2_sb, lw2_sb, dots_sb = state.pop(b)

                # ---- layer 0: h0^T [H, N] = W0d^T @ dots + [w0n; c_b] @ [norms; ones]
                h0_sb = actp.tile([128, N], tdt, tag="h0")
                h0_ps = h0pp.tile([128, N], f32, tag="h0ps")
                for half in range(2):
                    sl = slice(512 * half, 512 * (half + 1))
                    nc.tensor.matmul(
                        h0_ps[:, sl],
                        lw2_sb[:],
                        rhs2_sb[:, sl],
                        start=True,
                        stop=False,
                    )
                for c in range(NCHUNK):
                    lhsT = w0d_sb[:, 128 * c : 128 * (c + 1)]
                    for half in range(2):
                        nc.tensor.matmul(
                            h0_ps[:, 512 * half : 512 * (half + 1)],
                            lhsT,
                            dots_sb[:, N * c + 512 * half : N * c + 512 * (half + 1)],
                            start=False,
                            stop=(c == NCHUNK - 1),
                        )
                for half in range(2):
                    sl = slice(512 * half, 512 * (half + 1))
                    ltmp = actp.tile([128, 512], f32, tag="ltmp", name=f"ltmp0_{half}")
                    leaky_evict(h0_sb[:, sl], h0_ps[:, sl], ltmp[:])

                # ---- layer 1 in [i, H] layout:
                # h1[128c+p, h] lives at h1_ps[p, 128c+h].
                # bias: one broadcast matmul ones^T (x) tile(b1, 8)
                h1_ps = h1pp.tile([128, N], f32, tag="h1ps")
                for half in range(2):
                    sl = slice(512 * half, 512 * (half + 1))
                    nc.tensor.matmul(
                        h1_ps[:, sl],
                        ones_sb[:, 0:128],
                        b1t_sb[:, sl],
                        start=True,
                        stop=False,
                    )
                for c in range(NCHUNK):
                    nc.tensor.matmul(
                        h1_ps[:, 128 * c : 128 * (c + 1)],
                        h0_sb[:, 128 * c : 128 * (c + 1)],
                        w1_sb[:] if not tail_bf16 else w1_sb[:],
                        start=False,
                        stop=True,
                        skip_group_check=True,
                    )
                h1c_sb = actp.tile([128, N], tdt, tag="h1c")
                for half in range(2):
                    sl = slice(512 * half, 512 * (half + 1))
                    ltmp1 = actp.tile([128, 512], f32, tag="ltmp", name=f"ltmp1_{half}")
                    leaky_evict(h1c_sb[:, sl], h1_ps[:, sl], ltmp1[:])

                # ---- y_b [3, H] = x_b^T @ h1  (accumulate over 8 i-chunks)
                y_ps = h1pp.tile([3, H], f32, tag="h1ps", name=f"y_ps{b}")
                for c in range(NCHUNK):
                    nc.tensor.matmul(
                        y_ps[:],
                        xc_sb[:, 3 * c : 3 * (c + 1)],
                        h1c_sb[:, 128 * c : 128 * (c + 1)],
                        start=(c == 0),
                        stop=(c == NCHUNK - 1),
                    )
                y_sb = youtp.tile([3, H], f32, tag="y")
                nc.vector.tensor_copy(y_sb[:], y_ps[:])
                nc.sync.dma_start(out=y_d[b], in_=y_sb[:])

            # software-pipelined emission: gram(b) ahead of mlp(b-1) so the
            # ScalarE sqrt stream never starves while PE runs the MLP tail.
            for s in range(BPC + 1):
                if s < BPC:
                    emit_gram(s)
                if s >= 1:
                    emit_mlp(s - 1)

    nc.finalize()
    return nc


def _host_prep(x, u, W0, b0, W1, b1):
    """Build per-core input maps."""
    import ml_dtypes

    tnp = ml_dtypes.bfloat16 if TAIL_BF16 else np.float32
    xT = np.ascontiguousarray(x.transpose(0, 2, 1))  # [B, 3, N]
    xc = np.ascontiguousarray(
        x.reshape(B, N // 128, 128, 3).transpose(0, 2, 1, 3).reshape(B, 128, 24)
    ).astype(tnp)
    norms = np.sqrt((x.astype(np.float64) ** 2).sum(-1)).astype(np.float32)  # [B, N]
    rhs2 = np.stack([norms, np.ones_like(norms)], axis=1)  # [B, 2, N]
    cb = (u @ W0[:G] + b0).astype(np.float32)  # [B, H]
    w0n = np.broadcast_to(W0[G], (B, H)).astype(np.float32)
    lw2 = np.ascontiguousarray(np.stack([w0n, cb], axis=1))  # [B, 2, H]
    w0d = np.ascontiguousarray(
        W0[G + 1 :].reshape(N // 128, 128, H).transpose(1, 0, 2).reshape(128, N // 128 * H)
    )

    in_maps = []
    for c in range(N_CORES):
        sl = slice(BPC * c, BPC * (c + 1))
        in_maps.append(
            {
                "xT": np.ascontiguousarray(xT[sl]),
                "xc": np.ascontiguousarray(xc[sl]),
                "rhs2": np.ascontiguousarray(rhs2[sl]),
                "lw2": np.ascontiguousarray(lw2[sl]),
                "w0d": w0d,
                "w1": np.ascontiguousarray(W1).astype(tnp),
                "b1t": np.tile(b1, N // H)[None, :].astype(np.float32),
                "ones": np.ones((1, N), dtype=np.float32),
            }
        )
    return in_maps


def kernel(x, u, W0, b0, W1, b1, W2, b2, _run_kwargs=None):
    x = np.asarray(x, dtype=np.float32)
    u = np.asarray(u, dtype=np.float32)
    W0 = np.asarray(W0, dtype=np.float32)
    b0 = np.asarray(b0, dtype=np.float32)
    W1 = np.asarray(W1, dtype=np.float32)
    b1 = np.asarray(b1, dtype=np.float32)
    W2 = np.asarray(W2, dtype=np.float32)
    b2 = np.asarray(b2, dtype=np.float32)

    from concourse.bass_utils import run_bass_kernel_spmd

    if "nc" not in _cached:
        _cached["nc"] = _build_nc()
    nc = _cached["nc"]

    in_maps = _host_prep(x, u, W0, b0, W1, b1)
    kw = dict(_run_kwargs or {})
    res = run_bass_kernel_spmd(nc, in_maps, list(range(N_CORES)), **kw)
    _cached["last_results"] = res
    y = np.concatenate([r["y"] for r in res.results], axis=0)  # [B, 3, H]

    # host finish: out[b,o,d] = sum_h W2[h,o] y[b,d,h] / N + b2[o]*colsum_x[b,d]/N
    colsum = x.sum(axis=1)  # [B, 3]
    out = (
        np.einsum("ho,bdh->bod", W2.astype(np.float64), y.astype(np.float64))
        + b2.astype(np.float64)[None, :, None] * colsum.astype(np.float64)[:, None, :]
    ) / N
    return out.astype(np.float32)
